# revision 23
# baseline (speedup 1.0000x reference)
"""Trainium2 Bass kernel v3 for nn_BufferClassifier (B=32768, BUF=4096).

Changes vs v2 baseline (2.385 ms):
  - hid double-buffered: GEMM1 of sub-tile s+1 no longer serializes at the
    sub-tile boundary behind the WAR hazard on GEMM2(s)'s hid reads. The
    whole front-end (static DMA, RNN, GEMM1) of s+1 is interleaved into
    GEMM2(s)'s 32 m-group slots.
  - exp staging + output in bf16 (host upcasts to f32): frees SBUF for the
    hid double buffer and halves output DMA traffic.
"""
import numpy as np

import concourse.bass as bass  # noqa: F401
from concourse import bacc
import concourse.mybir as mybir
import concourse.tile as tile

B = 32768
BUF = 4096
H = 10
T = 10
NCORES = 8
BC = B // NCORES
BSUB = 512
NSUB = BC // BSUB
NM = BSUB // 128            # 4
NK = BUF // 128             # 32
NCLS = BUF // 512           # 8

F32R = mybir.dt.float32r
F32 = mybir.dt.float32
BF16 = mybir.dt.bfloat16
AF = mybir.ActivationFunctionType
AX = mybir.AxisListType
OP = mybir.AluOpType

_CACHE = {}


def _build(nrep=1):
    nc = bacc.Bacc(None, target_bir_lowering=False)
    d_rs = nc.dram_tensor("rs", [NSUB, 25, BSUB], BF16, kind="ExternalInput")
    d_rx = nc.dram_tensor("rx", [NSUB, 5, T, BSUB], F32R, kind="ExternalInput")
    d_Wrx = nc.dram_tensor("Wrx", [5, 50], F32R, kind="ExternalInput")
    d_Wrh = nc.dram_tensor("Wrh", [50, 50], F32R, kind="ExternalInput")
    d_rb = nc.dram_tensor("rb", [50, 1], F32, kind="ExternalInput")
    # whc rows 0-49: w_hidden RNN part; rows 50-74: w_hidden_static @ Ws
    d_whc = nc.dram_tensor("whc", [75, NK, 128], BF16, kind="ExternalInput")
    d_bh = nc.dram_tensor("bh", [128, NK], F32, kind="ExternalInput")
    # wff[c, p, k, n] = w_ff[c*512+n, k*128+p]
    d_wff = nc.dram_tensor("wff", [NCLS, 128, NK, 512], BF16,
                           kind="ExternalInput")
    # ebf[p, j] = exp(b_ff[j]) broadcast over partitions
    d_ebf = nc.dram_tensor("ebf", [128, BUF], BF16, kind="ExternalInput")
    d_out = nc.dram_tensor("out", [BC, BUF], BF16, kind="ExternalOutput")

    with tile.TileContext(nc) as tc:
        with tc.tile_pool(name="const", bufs=1) as cst, \
             tc.tile_pool(name="hidp", bufs=2) as hidp, \
             tc.tile_pool(name="featp", bufs=2) as featp, \
             tc.tile_pool(name="hp", bufs=2) as hp, \
             tc.tile_pool(name="rxp", bufs=2) as rxp, \
             tc.tile_pool(name="wffp", bufs=2) as wffp, \
             tc.tile_pool(name="expp", bufs=1) as expp, \
             tc.tile_pool(name="smp", bufs=2) as smp, \
             tc.tile_pool(name="ps_aux", bufs=2, space="PSUM") as ps_aux, \
             tc.tile_pool(name="ps_g1", bufs=2, space="PSUM") as ps_g1, \
             tc.tile_pool(name="ps_g2", bufs=1, space="PSUM") as ps_g2:

            # --- constants, loaded once ---
            Wrxt = cst.tile([5, 50], F32R, name="Wrxt")
            nc.sync.dma_start(out=Wrxt, in_=d_Wrx[:, :])
            Wrht = cst.tile([50, 50], F32R, name="Wrht")
            nc.sync.dma_start(out=Wrht, in_=d_Wrh[:, :])
            rbt = cst.tile([50, 1], F32, name="rbt")
            nc.sync.dma_start(out=rbt, in_=d_rb[:, :])
            bht = cst.tile([128, NK], F32, name="bht")
            nc.sync.dma_start(out=bht, in_=d_bh[:, :])
            whc_r = cst.tile([75, NK, 128], BF16, name="whc_r")
            nc.sync.dma_start(out=whc_r, in_=d_whc[:, :, :])
            ebft = cst.tile([128, BUF], BF16, name="ebft")
            nc.sync.dma_start(out=ebft, in_=d_ebf[:, :])

            def fe_static(s):
                """comb [75, BSUB] bf16: rows 0-49 <- final RNN tanh,
                rows 50-74 <- rs via DMA. Also allocates this sub-tile's
                hid^T buffers (double-buffered pool)."""
                comb = featp.tile([75, BSUB], BF16, tag="fs", name=f"comb_{s}")
                # ACT's HWDGE queue keeps these small latency-critical
                # transfers out of the 4MB w_ff stream on the SP queue.
                nc.scalar.dma_start(out=comb[50:75, :], in_=d_rs[s])
                hid = [hidp.tile([128, BSUB], BF16, tag=f"hid{j}",
                                 name=f"hid{j}_{s}") for j in range(NK)]
                return dict(comb=comb, h_prev=None, hid=hid)

            def fe_rnn_step(s, t, fe):
                rxt = rxp.tile([5, BSUB], F32R, tag="rxt", name=f"rxt_{s}_{t}")
                nc.scalar.dma_start(out=rxt, in_=d_rx[s, :, t, :])
                ph = ps_aux.tile([50, BSUB], F32, tag="psx", name=f"ph_{s}_{t}")
                nc.tensor.matmul(out=ph, lhsT=Wrxt[:], rhs=rxt[:],
                                 start=True, stop=(t == 0))
                if t > 0:
                    nc.tensor.matmul(out=ph, lhsT=Wrht[:], rhs=fe["h_prev"][:],
                                     start=False, stop=True)
                dst = fe["comb"][0:50, :] if t == T - 1 else hp.tile(
                    [50, BSUB], F32R, tag="h", name=f"h_{s}_{t}")
                nc.scalar.activation(out=dst, in_=ph, func=AF.Tanh, bias=rbt[:])
                fe["h_prev"] = dst

            def fe_gemm1_unit(s, j, fe):
                pg = ps_g1.tile([128, BSUB], F32, tag="psg1",
                                name=f"pg_{s}_{j}")
                nc.tensor.matmul(out=pg, lhsT=whc_r[:, j, :],
                                 rhs=fe["comb"][:], start=True, stop=True)
                nc.scalar.activation(out=fe["hid"][j], in_=pg, func=AF.Relu,
                                     bias=bht[:, j:j + 1])

            def body(fe):
                for s in range(NSUB):
                    cur = fe[0]
                    nxt = [None]
                    # next sub-tile's front-end, chopped into pieces that
                    # slot in between GEMM2 m-groups (32 slots available).
                    # s == NSUB-1 prepares sub-tile 0 again: every For_i
                    # iteration consumes identical data, so this pipelines
                    # the next repetition's prologue under this one's tail
                    # (for nrep=1 it is dead work, ~1.5% overhead).
                    pieces = []
                    sn = (s + 1) % NSUB

                    def _static(nxt=nxt, sn=sn):
                        nxt[0] = fe_static(sn)
                    pieces.append(_static)
                    for i in range(5):
                        def _rnn(i=i, nxt=nxt, sn=sn):
                            fe_rnn_step(sn, 2 * i, nxt[0])
                            fe_rnn_step(sn, 2 * i + 1, nxt[0])
                        pieces.append(_rnn)
                    for i in range(NK // 2):
                        def _g1(i=i, nxt=nxt, sn=sn):
                            fe_gemm1_unit(sn, 2 * i, nxt[0])
                            fe_gemm1_unit(sn, 2 * i + 1, nxt[0])
                        pieces.append(_g1)

                    def fe_piece(slot):
                        if slot < len(pieces):
                            pieces[slot]()

                    # --- GEMM2 + softmax for sub-tile s ---
                    exps = [expp.tile([128, BUF], BF16, tag=f"exp{m}",
                                      name=f"exp_{s}_{m}") for m in range(NM)]
                    pars = [smp.tile([128, NCLS], F32, tag=f"par{m}",
                                     name=f"par_{s}_{m}") for m in range(NM)]
                    for c in range(NCLS):
                        wf = wffp.tile([128, NK, 512], BF16, tag="wff",
                                       name=f"wf_{s}_{c}")
                        nc.sync.dma_start(out=wf, in_=d_wff[c])
                        pts = [ps_g2.tile([128, 512], F32, tag=f"psg2_{m}",
                                          name=f"pt_{s}_{c}_{m}")
                               for m in range(NM)]
                        for m in range(NM):
                            for k in range(NK):
                                nc.tensor.matmul(
                                    out=pts[m],
                                    lhsT=cur["hid"][k][:, m * 128:(m + 1) * 128],
                                    rhs=wf[:, k, :], start=(k == 0),
                                    stop=(k == NK - 1))
                            fe_piece(4 * c + m)
                            sl = slice(c * 512, (c + 1) * 512)
                            nc.scalar.activation(
                                out=exps[m][:, sl], in_=pts[m], func=AF.Exp)
                            nc.vector.scalar_tensor_tensor(
                                out=exps[m][:, sl], in0=exps[m][:, sl],
                                scalar=1.0, in1=ebft[:, sl],
                                op0=OP.bypass, op1=OP.mult,
                                accum_out=pars[m][:, c:c + 1])
                    for m in range(NM):
                        sm = smp.tile([128, 1], F32, tag=f"sum{m}",
                                      name=f"sum_{s}_{m}")
                        nc.vector.reduce_sum(out=sm, in_=pars[m][:], axis=AX.X)
                        rec = smp.tile([128, 1], F32, tag=f"rec{m}",
                                       name=f"rec_{s}_{m}")
                        nc.vector.reciprocal(rec, sm)
                        nc.vector.tensor_scalar(
                            out=exps[m][:], in0=exps[m][:], scalar1=rec[:],
                            scalar2=None, op0=OP.mult)
                        row0 = s * BSUB + m * 128
                        # ACT's HWDGE queue: the next sub-tile's first exp
                        # WARs on this transfer; on the SP queue it can sit
                        # behind an 11us w_ff prefetch right at the boundary.
                        nc.scalar.dma_start(out=d_out[row0:row0 + 128, :],
                                            in_=exps[m][:])
                    fe[0] = nxt[0]

            fe = [fe_static(0)]
            for t in range(T):
                fe_rnn_step(0, t, fe[0])
            for j in range(NK):
                fe_gemm1_unit(0, j, fe[0])
            if nrep == 1:
                body(fe)
            else:
                with tc.For_i(0, nrep, 1):
                    body(fe)
    nc.finalize()
    return nc


def _prep(inputs):
    import ml_dtypes
    f = np.float32
    bf = ml_dtypes.bfloat16
    inputs = {k: np.asarray(v, f) for k, v in inputs.items()}
    data = inputs["data"]
    idx1 = data[:, 1].astype(np.int32)
    idx2 = data[:, 2].astype(np.int32)

    rs = np.empty((25, B), f)
    for r in range(3):
        rs[r] = (idx1 == r)
    for r in range(4):
        rs[3 + r] = (idx2 == r)
    rs[7:24] = data[:, 3:20].T
    rs[24] = 1.0

    Ws = np.zeros((25, 90), f)
    Ws[0:3, 0:10] = inputs["emb_client"]
    Ws[3:7, 10:20] = inputs["emb_lastreq"]
    for i, nm in enumerate(["req", "seq", "tac", "tcl", "tl"]):
        Ws[7 + i, 20 + 10 * i:30 + 10 * i] = inputs[f"w_{nm}"][:, 0]
        Ws[24, 20 + 10 * i:30 + 10 * i] = inputs[f"b_{nm}"]
    Ws[12:23, 70:80] = inputs["w_mem"].T
    Ws[24, 70:80] = inputs["b_mem"]
    Ws[23, 80:90] = inputs["w_cpu"][:, 0]
    Ws[24, 80:90] = inputs["b_cpu"]

    rx = np.ascontiguousarray(
        data[:, 20:70].reshape(B, 5, T).transpose(1, 2, 0))  # [5, T, B]

    wih = [inputs["pw_wih"]] + [inputs["h_wih"]] * 4
    whh = [inputs["pw_whh"]] + [inputs["h_whh"]] * 4
    bi = [inputs["pw_bih"] + inputs["pw_bhh"]] + \
         [inputs["h_bih"] + inputs["h_bhh"]] * 4
    Wrx = np.zeros((5, 50), f)
    Wrh = np.zeros((50, 50), f)
    for j in range(5):
        Wrx[j, 10 * j:10 * j + 10] = wih[j][:, 0]
        Wrh[10 * j:10 * j + 10, 10 * j:10 * j + 10] = whh[j].T
    rb = np.concatenate(bi).astype(f).reshape(50, 1)

    wh = np.ascontiguousarray(inputs["w_hidden"].T)       # [140, 4096]
    # fold the static-feature projection: whc_static = Ws @ wh_static
    whc_static = (Ws.astype(np.float64) @ wh[0:90].astype(np.float64))
    whc = np.empty((75, BUF), np.float64)
    whc[0:50] = wh[90:140]
    whc[50:75] = whc_static
    whc = np.ascontiguousarray(whc.reshape(75, NK, 128)).astype(bf)
    bh = np.ascontiguousarray(inputs["b_hidden"].reshape(NK, 128).T)  # [128,NK]

    wt = np.ascontiguousarray(inputs["w_ff"].T)           # [feat, cls]
    wff = np.ascontiguousarray(
        wt.reshape(NK, 128, NCLS, 512).transpose(2, 1, 0, 3)).astype(bf)
    ebf = np.broadcast_to(
        np.exp(inputs["b_ff"]).astype(bf).reshape(1, BUF), (128, BUF))
    ebf = np.ascontiguousarray(ebf)

    shared = dict(Wrx=Wrx, Wrh=Wrh, rb=rb, whc=whc,
                  bh=bh, wff=wff, ebf=ebf)
    in_maps = []
    for c in range(NCORES):
        sl = slice(c * BC, (c + 1) * BC)
        rs_c = np.ascontiguousarray(
            rs[:, sl].reshape(25, NSUB, BSUB).transpose(1, 0, 2)).astype(bf)
        rx_c = np.ascontiguousarray(
            rx[:, :, sl].reshape(5, T, NSUB, BSUB).transpose(2, 0, 1, 3))
        in_maps.append(dict(rs=rs_c, rx=rx_c, **shared))
    return in_maps


def get_nc(nrep=1):
    if nrep not in _CACHE:
        _CACHE[nrep] = _build(nrep)
    return _CACHE[nrep]


def kernel(**inputs) -> np.ndarray:
    from concourse.bass_utils import run_bass_kernel_spmd
    nc = get_nc()
    in_maps = _prep(inputs)
    last = None
    for attempt in range(4):
        try:
            res = run_bass_kernel_spmd(nc, in_maps, core_ids=list(range(NCORES)))
            break
        except Exception as e:  # transient NRT device errors recover on retry
            last = e
            import time
            time.sleep(5 * (attempt + 1))
    else:
        raise last
    return np.concatenate(
        [res.results[c]["out"].astype(np.float32) for c in range(NCORES)],
        axis=0)
